# revision 18
# baseline (speedup 1.0000x reference)
"""KGNN head kernel for Trainium2 (Bass/Tile), 8-core data-parallel SPMD.

Computation (per batch b):
    score[g, n] = sum_d drug[b, g, d] * rel[b, 8g+n, d]         (n in 0..8)
    out[b, g, d] = sum_n score[g, n] * ent[b, 8g+n, d]

Layout: one SBUF partition holds one BATCH (two 128-batch blocks per core);
a tile covers 8 groups x 8 neighbors x 64 dims = 4096 elements of free dim.
Per-partition HBM runs are 16KiB for rel/ent and 2KiB for drug/out, so DMA
descriptors are large and sequential (~425GB/s effective when busy).

Per tile (pb, gt):
  - DVE tensor_tensor (rel * drug broadcast over n) -> prod bf16.
  - d-reduction: two bf16 2x fold-adds (64->32->16) + one tensor_reduce
    (16->1) -> score fp32 [128, gc*NN].
  - ACT materializes score_rep (score broadcast over d) in bf16 and casts
    ent fp32 -> bf16 (ACT never contends with DVE; GpSimd would).
  - DVE tensor_tensor w = ent_bf16 * score_rep in bf16 (2x packed), neighbor
    axis outermost, split into two halves so matmuls start earlier.
  - Sum over the 8 neighbor chunks: PSUM-accumulating bf16 matmuls with a
    constant bf16 identity as lhsT.
  - ACT copies PSUM -> SBUF and issues the output DMA from its own HWDGE
    queue (the in-order SP queue stays input-only, so loads never block
    behind an output DMA waiting on compute).
"""

import numpy as np

import concourse.bass as bass  # noqa: F401  (engine namespaces via nc)
import concourse.mybir as mybir
import concourse.tile as tile
from concourse import bacc
from concourse.bass_utils import run_bass_kernel_spmd
from concourse.masks import make_identity

F32 = mybir.dt.float32
BF16 = mybir.dt.bfloat16

N_CORES = 8
B_FULL = 2048
B_LOCAL = B_FULL // N_CORES  # 256
G = 64          # groups per sample
NN = 8          # neighbors per group
D = 64          # feature dim
S = G * NN      # 512 neighbor slots

PB = B_LOCAL // 128  # 128-batch partition blocks (2)
GT = 8               # group tiles
GC = G // GT         # groups per tile (8)
FD = GC * NN * D     # free dim per tile (4096)

CAST_DMA = False  # fp32->bf16 during input DMAs (SWDGE)


def _build_nc(b_local: int = B_LOCAL) -> "bacc.Bacc":
    pb_n = b_local // 128
    assert pb_n * 128 == b_local

    nc = bacc.Bacc("TRN2", target_bir_lowering=False, debug=False)

    drug_d = nc.dram_tensor("drug", [b_local, G, D], F32, kind="ExternalInput")
    rel_d = nc.dram_tensor("rel", [b_local, S, D], F32, kind="ExternalInput")
    ent_d = nc.dram_tensor("ent", [b_local, S, D], F32, kind="ExternalInput")
    out_d = nc.dram_tensor("out", [b_local, G, D], F32, kind="ExternalOutput")

    rel_v = rel_d[:].rearrange(
        "(pb p) (gt gc n) d -> pb gt p (gc n d)", pb=pb_n, gt=GT, gc=GC, n=NN
    )
    ent_v = ent_d[:].rearrange(
        "(pb p) (gt gc n) d -> pb gt p (gc n d)", pb=pb_n, gt=GT, gc=GC, n=NN
    )
    drug_v = drug_d[:].rearrange("(pb p) g d -> pb p (g d)", pb=pb_n)
    out_v = out_d[:].rearrange(
        "(pb p) (gt gc) d -> pb gt p (gc d)", pb=pb_n, gt=GT, gc=GC
    )

    in_dt = BF16 if CAST_DMA else F32
    in_dma = nc.gpsimd.dma_start if CAST_DMA else nc.sync.dma_start

    with tile.TileContext(nc) as tc:
        with (
            tc.tile_pool(name="const", bufs=1) as const_pool,
            tc.tile_pool(name="rel", bufs=3) as rel_pool,
            tc.tile_pool(name="ent", bufs=3) as ent_pool,
            tc.tile_pool(name="entb", bufs=2) as entb_pool,
            tc.tile_pool(name="drug", bufs=1) as drug_pool,
            tc.tile_pool(name="score", bufs=3) as score_pool,
            tc.tile_pool(name="srep", bufs=1) as srep_pool,
            tc.tile_pool(name="prod", bufs=2) as prod_pool,
            tc.tile_pool(name="fold", bufs=2) as fold_pool,
            tc.tile_pool(name="w", bufs=2) as w_pool,
            tc.tile_pool(name="outs", bufs=2) as out_pool,
            tc.tile_pool(name="psum", bufs=2, space="PSUM") as psum_pool,
        ):
            ident = const_pool.tile([128, 128], BF16)
            make_identity(nc, ident[:])

            drug_t = drug_pool.tile([128, pb_n * G * D], in_dt)
            drug_view = drug_t[:].rearrange(
                "p (pb g d) -> p pb g d", pb=pb_n, g=G
            )

            rel_ts, ent_ts = {}, {}

            def load_tile(t):
                pb, gt = divmod(t, GT)
                rel_ts[t] = rel_pool.tile([128, FD], in_dt, name="rel_t")
                in_dma(out=rel_ts[t][:], in_=rel_v[pb, gt])
                if t == 0:
                    # drug block 0 right after rel0 so tile-0 compute
                    # unblocks as early as possible; block 1 after ent0
                    in_dma(
                        out=drug_t[:, 0 : G * D], in_=drug_v[0]
                    )
                ent_ts[t] = ent_pool.tile([128, FD], in_dt, name="ent_t")
                in_dma(out=ent_ts[t][:], in_=ent_v[pb, gt])
                if t == 0 and pb_n > 1:
                    for pb2 in range(1, pb_n):
                        in_dma(
                            out=drug_t[:, pb2 * G * D : (pb2 + 1) * G * D],
                            in_=drug_v[pb2],
                        )

            def tail_half(pb, gt, h, rel_t, ent_t):
                """Last tile: process a gc=4 half with a short dependency
                chain (fp32 wmul, no srep/entcast) so the kernel tail
                drains faster. Reuses the main pools' tile names."""
                gcw = GC // 2
                fdw = gcw * NN * D  # 2048
                unw = gcw * NN      # 32
                lo = h * gcw
                rel_s = rel_t[:, lo * NN * D : (lo + gcw) * NN * D]
                ent_s = ent_t[:, lo * NN * D : (lo + gcw) * NN * D]
                prod_t = prod_pool.tile([128, FD], BF16, name="prod_t")
                nc.vector.tensor_tensor(
                    out=prod_t[:, :fdw].rearrange(
                        "p (gc n d) -> p gc n d", gc=gcw, n=NN
                    ),
                    in0=rel_s.rearrange("p (gc n d) -> p gc n d", gc=gcw, n=NN),
                    in1=drug_view[:, pb, gt * GC + lo : gt * GC + lo + gcw]
                    .unsqueeze(2)
                    .to_broadcast([128, gcw, NN, D]),
                    op=mybir.AluOpType.mult,
                )
                f1_t = fold_pool.tile(
                    [128, GC * NN * (D // 2 + D // 4)], BF16, name="f1_t"
                )
                f1 = f1_t[:, : unw * (D // 2)].rearrange(
                    "p (un x) -> p un x", un=unw
                )
                f2 = f1_t[
                    :, unw * (D // 2) : unw * (D // 2) + unw * (D // 4)
                ].rearrange("p (un x) -> p un x", un=unw)
                pv = prod_t[:, :fdw].rearrange("p (un d) -> p un d", un=unw)
                nc.vector.tensor_tensor(
                    out=f1, in0=pv[:, :, 0 : D // 2], in1=pv[:, :, D // 2 : D],
                    op=mybir.AluOpType.add,
                )
                nc.vector.tensor_tensor(
                    out=f2, in0=f1[:, :, 0 : D // 4], in1=f1[:, :, D // 4 : D // 2],
                    op=mybir.AluOpType.add,
                )
                score_t = score_pool.tile([128, GC * NN], F32, name="score_t")
                nc.vector.tensor_reduce(
                    out=score_t[:, :unw],
                    in_=f2,
                    axis=mybir.AxisListType.X,
                    op=mybir.AluOpType.add,
                )
                w_t = w_pool.tile([128, FD], BF16, name="w_t")
                nc.vector.tensor_tensor(
                    out=w_t[:, :fdw].rearrange(
                        "p (n gc d) -> p gc n d", n=NN, gc=gcw
                    ),
                    in0=ent_s.rearrange("p (gc n d) -> p gc n d", gc=gcw, n=NN),
                    in1=score_t[:, :unw]
                    .rearrange("p (gc n) -> p gc n", gc=gcw)
                    .unsqueeze(3)
                    .to_broadcast([128, gcw, NN, D]),
                    op=mybir.AluOpType.mult,
                )
                psum_t = psum_pool.tile([128, GC * D], F32, name="psum_t")
                for c in range(NN):
                    nc.tensor.matmul(
                        out=psum_t[:, : gcw * D],
                        lhsT=ident[:],
                        rhs=w_t[:, c * gcw * D : (c + 1) * gcw * D],
                        start=(c == 0),
                        stop=(c == NN - 1),
                    )
                out_t = out_pool.tile([128, GC * D], F32, name="out_t")
                nc.scalar.copy(out=out_t[:, : gcw * D], in_=psum_t[:, : gcw * D])
                nc.scalar.dma_start(
                    out=out_v[pb, gt][:, lo * D : (lo + gcw) * D],
                    in_=out_t[:, : gcw * D],
                )

            n_tiles = pb_n * GT
            load_tile(0)
            for t in range(n_tiles):
                pb, gt = divmod(t, GT)
                if t + 1 < n_tiles:
                    load_tile(t + 1)
                rel_t, ent_t = rel_ts.pop(t), ent_ts.pop(t)

                if t == n_tiles - 1:
                    tail_half(pb, gt, 0, rel_t, ent_t)
                    tail_half(pb, gt, 1, rel_t, ent_t)
                    continue

                # prod = rel * drug (broadcast over n), bf16
                prod_t = prod_pool.tile([128, FD], BF16)
                nc.vector.tensor_tensor(
                    out=prod_t[:].rearrange("p (gc n d) -> p gc n d", gc=GC, n=NN),
                    in0=rel_t[:].rearrange("p (gc n d) -> p gc n d", gc=GC, n=NN),
                    in1=drug_view[:, pb, gt * GC : (gt + 1) * GC]
                    .unsqueeze(2)
                    .to_broadcast([128, GC, NN, D]),
                    op=mybir.AluOpType.mult,
                )

                # d-reduction: bf16 2x folds 64->32->16, then reduce 16->1
                un = GC * NN  # 64 segments
                f1_t = fold_pool.tile([128, un * (D // 2) + un * (D // 4)], BF16)
                f1 = f1_t[:, : un * (D // 2)].rearrange(
                    "p (un h) -> p un h", un=un
                )
                f2 = f1_t[:, un * (D // 2) :].rearrange(
                    "p (un q) -> p un q", un=un
                )
                pv = prod_t[:].rearrange("p (un d) -> p un d", un=un)
                nc.vector.tensor_tensor(
                    out=f1, in0=pv[:, :, 0 : D // 2], in1=pv[:, :, D // 2 : D],
                    op=mybir.AluOpType.add,
                )
                nc.vector.tensor_tensor(
                    out=f2, in0=f1[:, :, 0 : D // 4], in1=f1[:, :, D // 4 : D // 2],
                    op=mybir.AluOpType.add,
                )
                score_t = score_pool.tile([128, un], F32)
                nc.vector.tensor_reduce(
                    out=score_t[:],
                    in_=f2,
                    axis=mybir.AxisListType.X,
                    op=mybir.AluOpType.add,
                )

                # score_rep[gc, n, d] = score[gc, n]  (ACT, bf16 out)
                srep_t = srep_pool.tile([128, FD], BF16)
                nc.scalar.copy(
                    out=srep_t[:].rearrange(
                        "p (gc n d) -> p gc n d", gc=GC, n=NN
                    ),
                    in_=score_t[:]
                    .rearrange("p (gc n) -> p gc n", gc=GC)
                    .unsqueeze(3)
                    .to_broadcast([128, GC, NN, D]),
                )

                # ent cast fp32 -> bf16 on ACT (enables DVE 2x wmul)
                entb_t = entb_pool.tile([128, FD], BF16)
                nc.scalar.copy(out=entb_t[:], in_=ent_t[:])

                # w[n, gc, d] = score_rep * ent, bf16 2x, in two n-halves so
                # the matmuls overlap the second half
                w_t = w_pool.tile([128, FD], BF16)
                psum_t = psum_pool.tile([128, GC * D], F32)
                out_t = out_pool.tile([128, GC * D], F32)
                wv = w_t[:].rearrange("p (n gc d) -> p n gc d", n=NN, gc=GC)
                ev = entb_t[:].rearrange("p (gc n d) -> p gc n d", gc=GC, n=NN)
                sv = srep_t[:].rearrange("p (gc n d) -> p gc n d", gc=GC, n=NN)
                half = NN // 2
                for h in range(2):
                    nlo, nhi = h * half, (h + 1) * half
                    nc.vector.tensor_tensor(
                        out=wv[:, nlo:nhi].rearrange("p n gc d -> p gc n d"),
                        in0=ev[:, :, nlo:nhi],
                        in1=sv[:, :, nlo:nhi],
                        op=mybir.AluOpType.mult,
                    )
                    for c in range(nlo, nhi):
                        nc.tensor.matmul(
                            out=psum_t[:],
                            lhsT=ident[:],
                            rhs=w_t[:, c * GC * D : (c + 1) * GC * D],
                            start=(c == 0),
                            stop=(c == NN - 1),
                        )

                nc.scalar.copy(out=out_t[:], in_=psum_t[:])
                nc.scalar.dma_start(out=out_v[pb, gt], in_=out_t[:])

    nc.compile()
    return nc


_NC_CACHE: dict = {}


def _get_nc(b_local: int = B_LOCAL):
    if b_local not in _NC_CACHE:
        _NC_CACHE[b_local] = _build_nc(b_local)
    return _NC_CACHE[b_local]


def run_sharded(drug, rel, ent, trace: bool = False):
    """Shard batch dim across the 8 cores, run, gather. Returns
    (full output [B, G, D], BassKernelResults)."""
    drug = np.ascontiguousarray(np.asarray(drug, dtype=np.float32))
    rel = np.ascontiguousarray(np.asarray(rel, dtype=np.float32))
    ent = np.ascontiguousarray(np.asarray(ent, dtype=np.float32))
    b = drug.shape[0]
    nb = b // N_CORES
    assert nb * N_CORES == b
    nc = _get_nc(nb)
    in_maps = [
        {
            "drug": np.ascontiguousarray(drug[i * nb : (i + 1) * nb]),
            "rel": np.ascontiguousarray(rel[i * nb : (i + 1) * nb]),
            "ent": np.ascontiguousarray(ent[i * nb : (i + 1) * nb]),
        }
        for i in range(N_CORES)
    ]
    last_exc = None
    for attempt in range(3):
        try:
            res = run_bass_kernel_spmd(nc, in_maps, list(range(N_CORES)), trace=trace)
            break
        except Exception as exc:  # transient device-unrecoverable states
            last_exc = exc
            import time

            time.sleep(10 * (attempt + 1))
    else:
        raise last_exc
    out = np.concatenate([res.results[i]["out"] for i in range(N_CORES)], axis=0)
    return out, res


def kernel(drug, rel, ent):
    out, _ = run_sharded(drug, rel, ent, trace=False)
    return out


# revision 19
# speedup vs baseline: 1.1786x; 1.1786x over previous
"""KGNN head kernel for Trainium2 (Bass/Tile), 8-core data-parallel SPMD.

Computation (per batch b):
    score[g, n] = sum_d drug[b, g, d] * rel[b, 8g+n, d]         (n in 0..8)
    out[b, g, d] = sum_n score[g, n] * ent[b, 8g+n, d]

Layout: one SBUF partition holds one BATCH (two 128-batch blocks per core);
a tile covers 8 groups x 8 neighbors x 64 dims = 4096 elements of free dim.
Per-partition HBM runs are 16KiB for rel/ent and 2KiB for drug/out, so DMA
descriptors are large and sequential (~425GB/s effective when busy).

Per tile (pb, gt):
  - DVE tensor_tensor (rel * drug broadcast over n) -> prod bf16.
  - d-reduction: two bf16 2x fold-adds (64->32->16) + one tensor_reduce
    (16->1) -> score fp32 [128, gc*NN].
  - ACT materializes score_rep (score broadcast over d) in bf16 and casts
    ent fp32 -> bf16 (ACT never contends with DVE; GpSimd would).
  - DVE tensor_tensor w = ent_bf16 * score_rep in bf16 (2x packed), neighbor
    axis outermost, split into two halves so matmuls start earlier.
  - Sum over the 8 neighbor chunks: PSUM-accumulating bf16 matmuls with a
    constant bf16 identity as lhsT.
  - ACT copies PSUM -> SBUF and issues the output DMA from its own HWDGE
    queue (the in-order SP queue stays input-only, so loads never block
    behind an output DMA waiting on compute).
"""

import numpy as np

import concourse.bass as bass  # noqa: F401  (engine namespaces via nc)
import concourse.mybir as mybir
import concourse.tile as tile
from concourse import bacc
from concourse.bass_utils import run_bass_kernel_spmd
from concourse.masks import make_identity

F32 = mybir.dt.float32
BF16 = mybir.dt.bfloat16

N_CORES = 8
B_FULL = 2048
B_LOCAL = B_FULL // N_CORES  # 256
G = 64          # groups per sample
NN = 8          # neighbors per group
D = 64          # feature dim
S = G * NN      # 512 neighbor slots

PB = B_LOCAL // 128  # 128-batch partition blocks (2)
GT = 8               # group tiles
GC = G // GT         # groups per tile (8)
FD = GC * NN * D     # free dim per tile (4096)

CAST_DMA = False  # fp32->bf16 during input DMAs (SWDGE)


def _build_nc(b_local: int = B_LOCAL) -> "bacc.Bacc":
    pb_n = b_local // 128
    assert pb_n * 128 == b_local

    nc = bacc.Bacc("TRN2", target_bir_lowering=False, debug=False)

    drug_d = nc.dram_tensor("drug", [b_local, G, D], F32, kind="ExternalInput")
    rel_d = nc.dram_tensor("rel", [b_local, S, D], F32, kind="ExternalInput")
    ent_d = nc.dram_tensor("ent", [b_local, S, D], F32, kind="ExternalInput")
    out_d = nc.dram_tensor("out", [b_local, G, D], F32, kind="ExternalOutput")

    rel_v = rel_d[:].rearrange(
        "(pb p) (gt gc n) d -> pb gt p (gc n d)", pb=pb_n, gt=GT, gc=GC, n=NN
    )
    ent_v = ent_d[:].rearrange(
        "(pb p) (gt gc n) d -> pb gt p (gc n d)", pb=pb_n, gt=GT, gc=GC, n=NN
    )
    drug_v = drug_d[:].rearrange("(pb p) g d -> pb p (g d)", pb=pb_n)
    out_v = out_d[:].rearrange(
        "(pb p) (gt gc) d -> pb gt p (gc d)", pb=pb_n, gt=GT, gc=GC
    )

    in_dt = BF16 if CAST_DMA else F32
    in_dma = nc.gpsimd.dma_start if CAST_DMA else nc.sync.dma_start

    with tile.TileContext(nc) as tc:
        with (
            tc.tile_pool(name="const", bufs=1) as const_pool,
            tc.tile_pool(name="rel", bufs=4) as rel_pool,
            tc.tile_pool(name="ent", bufs=3) as ent_pool,
            tc.tile_pool(name="entb", bufs=1) as entb_pool,
            tc.tile_pool(name="drug", bufs=1) as drug_pool,
            tc.tile_pool(name="score", bufs=2) as score_pool,
            tc.tile_pool(name="srep", bufs=1) as srep_pool,
            tc.tile_pool(name="prod", bufs=1) as prod_pool,
            tc.tile_pool(name="fold", bufs=1) as fold_pool,
            tc.tile_pool(name="w", bufs=2) as w_pool,
            tc.tile_pool(name="outs", bufs=2) as out_pool,
            tc.tile_pool(name="psum", bufs=2, space="PSUM") as psum_pool,
        ):
            ident = const_pool.tile([128, 128], BF16)
            make_identity(nc, ident[:])

            drug_t = drug_pool.tile([128, pb_n * G * D], in_dt)
            drug_view = drug_t[:].rearrange(
                "p (pb g d) -> p pb g d", pb=pb_n, g=G
            )

            rel_ts, ent_ts = {}, {}

            def load_tile(t):
                pb, gt = divmod(t, GT)
                rel_ts[t] = rel_pool.tile([128, FD], in_dt, name="rel_t")
                in_dma(out=rel_ts[t][:], in_=rel_v[pb, gt])
                if t == 0:
                    # drug block 0 right after rel0 so tile-0 compute
                    # unblocks as early as possible; block 1 after ent0
                    in_dma(
                        out=drug_t[:, 0 : G * D], in_=drug_v[0]
                    )
                ent_ts[t] = ent_pool.tile([128, FD], in_dt, name="ent_t")
                in_dma(out=ent_ts[t][:], in_=ent_v[pb, gt])
                if t == 0 and pb_n > 1:
                    for pb2 in range(1, pb_n):
                        in_dma(
                            out=drug_t[:, pb2 * G * D : (pb2 + 1) * G * D],
                            in_=drug_v[pb2],
                        )

            def tail_half(pb, gt, h, rel_t, ent_t):
                """Last tile: process a gc=4 half with a short dependency
                chain (fp32 wmul, no srep/entcast) so the kernel tail
                drains faster. Reuses the main pools' tile names."""
                gcw = GC // 2
                fdw = gcw * NN * D  # 2048
                unw = gcw * NN      # 32
                lo = h * gcw
                rel_s = rel_t[:, lo * NN * D : (lo + gcw) * NN * D]
                ent_s = ent_t[:, lo * NN * D : (lo + gcw) * NN * D]
                prod_t = prod_pool.tile([128, FD], BF16, name="prod_t")
                nc.vector.tensor_tensor(
                    out=prod_t[:, :fdw].rearrange(
                        "p (gc n d) -> p gc n d", gc=gcw, n=NN
                    ),
                    in0=rel_s.rearrange("p (gc n d) -> p gc n d", gc=gcw, n=NN),
                    in1=drug_view[:, pb, gt * GC + lo : gt * GC + lo + gcw]
                    .unsqueeze(2)
                    .to_broadcast([128, gcw, NN, D]),
                    op=mybir.AluOpType.mult,
                )
                f1_t = fold_pool.tile(
                    [128, GC * NN * (D // 2 + D // 4)], BF16, name="f1_t"
                )
                f1 = f1_t[:, : unw * (D // 2)].rearrange(
                    "p (un x) -> p un x", un=unw
                )
                f2 = f1_t[
                    :, unw * (D // 2) : unw * (D // 2) + unw * (D // 4)
                ].rearrange("p (un x) -> p un x", un=unw)
                pv = prod_t[:, :fdw].rearrange("p (un d) -> p un d", un=unw)
                nc.vector.tensor_tensor(
                    out=f1, in0=pv[:, :, 0 : D // 2], in1=pv[:, :, D // 2 : D],
                    op=mybir.AluOpType.add,
                )
                nc.vector.tensor_tensor(
                    out=f2, in0=f1[:, :, 0 : D // 4], in1=f1[:, :, D // 4 : D // 2],
                    op=mybir.AluOpType.add,
                )
                score_t = score_pool.tile([128, GC * NN], F32, name="score_t")
                nc.vector.tensor_reduce(
                    out=score_t[:, :unw],
                    in_=f2,
                    axis=mybir.AxisListType.X,
                    op=mybir.AluOpType.add,
                )
                w_t = w_pool.tile([128, FD], BF16, name="w_t")
                nc.vector.tensor_tensor(
                    out=w_t[:, :fdw].rearrange(
                        "p (n gc d) -> p gc n d", n=NN, gc=gcw
                    ),
                    in0=ent_s.rearrange("p (gc n d) -> p gc n d", gc=gcw, n=NN),
                    in1=score_t[:, :unw]
                    .rearrange("p (gc n) -> p gc n", gc=gcw)
                    .unsqueeze(3)
                    .to_broadcast([128, gcw, NN, D]),
                    op=mybir.AluOpType.mult,
                )
                psum_t = psum_pool.tile([128, GC * D], F32, name="psum_t")
                for c in range(NN):
                    nc.tensor.matmul(
                        out=psum_t[:, : gcw * D],
                        lhsT=ident[:],
                        rhs=w_t[:, c * gcw * D : (c + 1) * gcw * D],
                        start=(c == 0),
                        stop=(c == NN - 1),
                    )
                out_t = out_pool.tile([128, GC * D], F32, name="out_t")
                nc.scalar.copy(out=out_t[:, : gcw * D], in_=psum_t[:, : gcw * D])
                nc.scalar.dma_start(
                    out=out_v[pb, gt][:, lo * D : (lo + gcw) * D],
                    in_=out_t[:, : gcw * D],
                )

            n_tiles = pb_n * GT
            load_tile(0)
            load_tile(1)
            for t in range(n_tiles):
                pb, gt = divmod(t, GT)
                if t + 2 < n_tiles:
                    load_tile(t + 2)
                rel_t, ent_t = rel_ts.pop(t), ent_ts.pop(t)

                if t == n_tiles - 1:
                    tail_half(pb, gt, 0, rel_t, ent_t)
                    tail_half(pb, gt, 1, rel_t, ent_t)
                    continue

                # prod = rel * drug (broadcast over n), bf16
                prod_t = prod_pool.tile([128, FD], BF16)
                nc.vector.tensor_tensor(
                    out=prod_t[:].rearrange("p (gc n d) -> p gc n d", gc=GC, n=NN),
                    in0=rel_t[:].rearrange("p (gc n d) -> p gc n d", gc=GC, n=NN),
                    in1=drug_view[:, pb, gt * GC : (gt + 1) * GC]
                    .unsqueeze(2)
                    .to_broadcast([128, GC, NN, D]),
                    op=mybir.AluOpType.mult,
                )

                # d-reduction: bf16 2x folds 64->32->16, then reduce 16->1
                un = GC * NN  # 64 segments
                f1_t = fold_pool.tile([128, un * (D // 2) + un * (D // 4)], BF16)
                f1 = f1_t[:, : un * (D // 2)].rearrange(
                    "p (un h) -> p un h", un=un
                )
                f2 = f1_t[:, un * (D // 2) :].rearrange(
                    "p (un q) -> p un q", un=un
                )
                pv = prod_t[:].rearrange("p (un d) -> p un d", un=un)
                nc.vector.tensor_tensor(
                    out=f1, in0=pv[:, :, 0 : D // 2], in1=pv[:, :, D // 2 : D],
                    op=mybir.AluOpType.add,
                )
                nc.vector.tensor_tensor(
                    out=f2, in0=f1[:, :, 0 : D // 4], in1=f1[:, :, D // 4 : D // 2],
                    op=mybir.AluOpType.add,
                )
                score_t = score_pool.tile([128, un], F32)
                nc.vector.tensor_reduce(
                    out=score_t[:],
                    in_=f2,
                    axis=mybir.AxisListType.X,
                    op=mybir.AluOpType.add,
                )

                # score_rep[gc, n, d] = score[gc, n]  (ACT, bf16 out)
                srep_t = srep_pool.tile([128, FD], BF16)
                nc.scalar.copy(
                    out=srep_t[:].rearrange(
                        "p (gc n d) -> p gc n d", gc=GC, n=NN
                    ),
                    in_=score_t[:]
                    .rearrange("p (gc n) -> p gc n", gc=GC)
                    .unsqueeze(3)
                    .to_broadcast([128, GC, NN, D]),
                )

                # ent cast fp32 -> bf16 on ACT (enables DVE 2x wmul)
                entb_t = entb_pool.tile([128, FD], BF16)
                nc.scalar.copy(out=entb_t[:], in_=ent_t[:])

                # w[n, gc, d] = score_rep * ent, bf16 2x, in two n-halves so
                # the matmuls overlap the second half
                w_t = w_pool.tile([128, FD], BF16)
                psum_t = psum_pool.tile([128, GC * D], F32)
                out_t = out_pool.tile([128, GC * D], F32)
                wv = w_t[:].rearrange("p (n gc d) -> p n gc d", n=NN, gc=GC)
                ev = entb_t[:].rearrange("p (gc n d) -> p gc n d", gc=GC, n=NN)
                sv = srep_t[:].rearrange("p (gc n d) -> p gc n d", gc=GC, n=NN)
                half = NN // 2
                for h in range(2):
                    nlo, nhi = h * half, (h + 1) * half
                    nc.vector.tensor_tensor(
                        out=wv[:, nlo:nhi].rearrange("p n gc d -> p gc n d"),
                        in0=ev[:, :, nlo:nhi],
                        in1=sv[:, :, nlo:nhi],
                        op=mybir.AluOpType.mult,
                    )
                    for c in range(nlo, nhi):
                        nc.tensor.matmul(
                            out=psum_t[:],
                            lhsT=ident[:],
                            rhs=w_t[:, c * GC * D : (c + 1) * GC * D],
                            start=(c == 0),
                            stop=(c == NN - 1),
                        )

                nc.scalar.copy(out=out_t[:], in_=psum_t[:])
                nc.scalar.dma_start(out=out_v[pb, gt], in_=out_t[:])

    nc.compile()
    return nc


_NC_CACHE: dict = {}


def _get_nc(b_local: int = B_LOCAL):
    if b_local not in _NC_CACHE:
        _NC_CACHE[b_local] = _build_nc(b_local)
    return _NC_CACHE[b_local]


def run_sharded(drug, rel, ent, trace: bool = False):
    """Shard batch dim across the 8 cores, run, gather. Returns
    (full output [B, G, D], BassKernelResults)."""
    drug = np.ascontiguousarray(np.asarray(drug, dtype=np.float32))
    rel = np.ascontiguousarray(np.asarray(rel, dtype=np.float32))
    ent = np.ascontiguousarray(np.asarray(ent, dtype=np.float32))
    b = drug.shape[0]
    nb = b // N_CORES
    assert nb * N_CORES == b
    nc = _get_nc(nb)
    in_maps = [
        {
            "drug": np.ascontiguousarray(drug[i * nb : (i + 1) * nb]),
            "rel": np.ascontiguousarray(rel[i * nb : (i + 1) * nb]),
            "ent": np.ascontiguousarray(ent[i * nb : (i + 1) * nb]),
        }
        for i in range(N_CORES)
    ]
    last_exc = None
    for attempt in range(3):
        try:
            res = run_bass_kernel_spmd(nc, in_maps, list(range(N_CORES)), trace=trace)
            break
        except Exception as exc:  # transient device-unrecoverable states
            last_exc = exc
            import time

            time.sleep(10 * (attempt + 1))
    else:
        raise last_exc
    out = np.concatenate([res.results[i]["out"] for i in range(N_CORES)], axis=0)
    return out, res


def kernel(drug, rel, ent):
    out, _ = run_sharded(drug, rel, ent, trace=False)
    return out


# revision 20
# speedup vs baseline: 1.1923x; 1.0117x over previous
"""KGNN head kernel for Trainium2 (Bass/Tile), 8-core data-parallel SPMD.

Computation (per batch b):
    score[g, n] = sum_d drug[b, g, d] * rel[b, 8g+n, d]         (n in 0..8)
    out[b, g, d] = sum_n score[g, n] * ent[b, 8g+n, d]

Layout: one SBUF partition holds one BATCH (two 128-batch blocks per core);
a tile covers 8 groups x 8 neighbors x 64 dims = 4096 elements of free dim.
Per-partition HBM runs are 16KiB for rel/ent and 2KiB for drug/out, so DMA
descriptors are large and sequential (~425GB/s effective when busy).

Per tile (pb, gt):
  - DVE tensor_tensor (rel * drug broadcast over n) -> prod bf16.
  - d-reduction: two bf16 2x fold-adds (64->32->16) + one tensor_reduce
    (16->1) -> score fp32 [128, gc*NN].
  - ACT materializes score_rep (score broadcast over d) in bf16 and casts
    ent fp32 -> bf16 (ACT never contends with DVE; GpSimd would).
  - DVE tensor_tensor w = ent_bf16 * score_rep in bf16 (2x packed), neighbor
    axis outermost, split into two halves so matmuls start earlier.
  - Sum over the 8 neighbor chunks: PSUM-accumulating bf16 matmuls with a
    constant bf16 identity as lhsT.
  - ACT copies PSUM -> SBUF and issues the output DMA from its own HWDGE
    queue (the in-order SP queue stays input-only, so loads never block
    behind an output DMA waiting on compute).
"""

import numpy as np

import concourse.bass as bass  # noqa: F401  (engine namespaces via nc)
import concourse.mybir as mybir
import concourse.tile as tile
from concourse import bacc
from concourse.bass_utils import run_bass_kernel_spmd
from concourse.masks import make_identity

F32 = mybir.dt.float32
BF16 = mybir.dt.bfloat16

N_CORES = 8
B_FULL = 2048
B_LOCAL = B_FULL // N_CORES  # 256
G = 64          # groups per sample
NN = 8          # neighbors per group
D = 64          # feature dim
S = G * NN      # 512 neighbor slots

PB = B_LOCAL // 128  # 128-batch partition blocks (2)
GT = 8               # group tiles
GC = G // GT         # groups per tile (8)
FD = GC * NN * D     # free dim per tile (4096)

CAST_DMA = False  # fp32->bf16 during input DMAs (SWDGE)


def _build_nc(b_local: int = B_LOCAL) -> "bacc.Bacc":
    pb_n = b_local // 128
    assert pb_n * 128 == b_local

    nc = bacc.Bacc("TRN2", target_bir_lowering=False, debug=False)

    drug_d = nc.dram_tensor("drug", [b_local, G, D], F32, kind="ExternalInput")
    rel_d = nc.dram_tensor("rel", [b_local, S, D], F32, kind="ExternalInput")
    ent_d = nc.dram_tensor("ent", [b_local, S, D], F32, kind="ExternalInput")
    out_d = nc.dram_tensor("out", [b_local, G, D], F32, kind="ExternalOutput")

    rel_v = rel_d[:].rearrange(
        "(pb p) (gt gc n) d -> pb gt p (gc n d)", pb=pb_n, gt=GT, gc=GC, n=NN
    )
    ent_v = ent_d[:].rearrange(
        "(pb p) (gt gc n) d -> pb gt p (gc n d)", pb=pb_n, gt=GT, gc=GC, n=NN
    )
    drug_v = drug_d[:].rearrange("(pb p) g d -> pb p (g d)", pb=pb_n)
    out_v = out_d[:].rearrange(
        "(pb p) (gt gc) d -> pb gt p (gc d)", pb=pb_n, gt=GT, gc=GC
    )

    in_dt = BF16 if CAST_DMA else F32
    in_dma = nc.gpsimd.dma_start if CAST_DMA else nc.sync.dma_start

    with tile.TileContext(nc) as tc:
        with (
            tc.tile_pool(name="const", bufs=1) as const_pool,
            tc.tile_pool(name="rel", bufs=3) as rel_pool,
            tc.tile_pool(name="ent", bufs=3) as ent_pool,
            tc.tile_pool(name="entb", bufs=2) as entb_pool,
            tc.tile_pool(name="drug", bufs=1) as drug_pool,
            tc.tile_pool(name="score", bufs=3) as score_pool,
            tc.tile_pool(name="srep", bufs=1) as srep_pool,
            tc.tile_pool(name="prod", bufs=2) as prod_pool,
            tc.tile_pool(name="fold", bufs=2) as fold_pool,
            tc.tile_pool(name="w", bufs=2) as w_pool,
            tc.tile_pool(name="outs", bufs=2) as out_pool,
            tc.tile_pool(name="psum", bufs=2, space="PSUM") as psum_pool,
        ):
            ident = const_pool.tile([128, 128], BF16)
            make_identity(nc, ident[:])

            drug_t = drug_pool.tile([128, pb_n * G * D], in_dt)
            drug_view = drug_t[:].rearrange(
                "p (pb g d) -> p pb g d", pb=pb_n, g=G
            )

            rel_ts, ent_ts = {}, {}

            def load_tile(t):
                pb, gt = divmod(t, GT)
                rel_ts[t] = rel_pool.tile([128, FD], in_dt, name="rel_t")
                in_dma(out=rel_ts[t][:], in_=rel_v[pb, gt])
                if t == 0:
                    # drug block 0 right after rel0 so tile-0 compute
                    # unblocks as early as possible; block 1 after ent0
                    in_dma(
                        out=drug_t[:, 0 : G * D], in_=drug_v[0]
                    )
                ent_ts[t] = ent_pool.tile([128, FD], in_dt, name="ent_t")
                in_dma(out=ent_ts[t][:], in_=ent_v[pb, gt])
                if t == 0 and pb_n > 1:
                    for pb2 in range(1, pb_n):
                        in_dma(
                            out=drug_t[:, pb2 * G * D : (pb2 + 1) * G * D],
                            in_=drug_v[pb2],
                        )

            def tail_half(pb, gt, h, rel_t, ent_t):
                """Last tile: process a gc=4 half with a short dependency
                chain (fp32 wmul, no srep/entcast) so the kernel tail
                drains faster. Reuses the main pools' tile names."""
                gcw = GC // 2
                fdw = gcw * NN * D  # 2048
                unw = gcw * NN      # 32
                lo = h * gcw
                rel_s = rel_t[:, lo * NN * D : (lo + gcw) * NN * D]
                ent_s = ent_t[:, lo * NN * D : (lo + gcw) * NN * D]
                prod_t = prod_pool.tile([128, FD], BF16, name="prod_t")
                nc.vector.tensor_tensor(
                    out=prod_t[:, :fdw].rearrange(
                        "p (gc n d) -> p gc n d", gc=gcw, n=NN
                    ),
                    in0=rel_s.rearrange("p (gc n d) -> p gc n d", gc=gcw, n=NN),
                    in1=drug_view[:, pb, gt * GC + lo : gt * GC + lo + gcw]
                    .unsqueeze(2)
                    .to_broadcast([128, gcw, NN, D]),
                    op=mybir.AluOpType.mult,
                )
                f1_t = fold_pool.tile(
                    [128, GC * NN * (D // 2 + D // 4)], BF16, name="f1_t"
                )
                f1 = f1_t[:, : unw * (D // 2)].rearrange(
                    "p (un x) -> p un x", un=unw
                )
                f2 = f1_t[
                    :, unw * (D // 2) : unw * (D // 2) + unw * (D // 4)
                ].rearrange("p (un x) -> p un x", un=unw)
                pv = prod_t[:, :fdw].rearrange("p (un d) -> p un d", un=unw)
                nc.vector.tensor_tensor(
                    out=f1, in0=pv[:, :, 0 : D // 2], in1=pv[:, :, D // 2 : D],
                    op=mybir.AluOpType.add,
                )
                nc.vector.tensor_tensor(
                    out=f2, in0=f1[:, :, 0 : D // 4], in1=f1[:, :, D // 4 : D // 2],
                    op=mybir.AluOpType.add,
                )
                score_t = score_pool.tile([128, GC * NN], F32, name="score_t")
                nc.vector.tensor_reduce(
                    out=score_t[:, :unw],
                    in_=f2,
                    axis=mybir.AxisListType.X,
                    op=mybir.AluOpType.add,
                )
                w_t = w_pool.tile([128, FD], BF16, name="w_t")
                nc.vector.tensor_tensor(
                    out=w_t[:, :fdw].rearrange(
                        "p (n gc d) -> p gc n d", n=NN, gc=gcw
                    ),
                    in0=ent_s.rearrange("p (gc n d) -> p gc n d", gc=gcw, n=NN),
                    in1=score_t[:, :unw]
                    .rearrange("p (gc n) -> p gc n", gc=gcw)
                    .unsqueeze(3)
                    .to_broadcast([128, gcw, NN, D]),
                    op=mybir.AluOpType.mult,
                )
                psum_t = psum_pool.tile([128, GC * D], F32, name="psum_t")
                for c in range(NN):
                    nc.tensor.matmul(
                        out=psum_t[:, : gcw * D],
                        lhsT=ident[:],
                        rhs=w_t[:, c * gcw * D : (c + 1) * gcw * D],
                        start=(c == 0),
                        stop=(c == NN - 1),
                    )
                out_t = out_pool.tile([128, GC * D], F32, name="out_t")
                nc.scalar.copy(out=out_t[:, : gcw * D], in_=psum_t[:, : gcw * D])
                nc.scalar.dma_start(
                    out=out_v[pb, gt][:, lo * D : (lo + gcw) * D],
                    in_=out_t[:, : gcw * D],
                )

            n_tiles = pb_n * GT
            load_tile(0)
            for t in range(n_tiles):
                pb, gt = divmod(t, GT)
                if t + 1 < n_tiles:
                    load_tile(t + 1)
                rel_t, ent_t = rel_ts.pop(t), ent_ts.pop(t)

                if t == n_tiles - 1:
                    tail_half(pb, gt, 0, rel_t, ent_t)
                    tail_half(pb, gt, 1, rel_t, ent_t)
                    continue

                # prod = rel * drug (broadcast over n), bf16
                prod_t = prod_pool.tile([128, FD], BF16)
                nc.vector.tensor_tensor(
                    out=prod_t[:].rearrange("p (gc n d) -> p gc n d", gc=GC, n=NN),
                    in0=rel_t[:].rearrange("p (gc n d) -> p gc n d", gc=GC, n=NN),
                    in1=drug_view[:, pb, gt * GC : (gt + 1) * GC]
                    .unsqueeze(2)
                    .to_broadcast([128, GC, NN, D]),
                    op=mybir.AluOpType.mult,
                )

                # d-reduction: bf16 2x folds 64->32->16, then reduce 16->1
                un = GC * NN  # 64 segments
                f1_t = fold_pool.tile([128, un * (D // 2) + un * (D // 4)], BF16)
                f1 = f1_t[:, : un * (D // 2)].rearrange(
                    "p (un h) -> p un h", un=un
                )
                f2 = f1_t[:, un * (D // 2) :].rearrange(
                    "p (un q) -> p un q", un=un
                )
                pv = prod_t[:].rearrange("p (un d) -> p un d", un=un)
                nc.vector.tensor_tensor(
                    out=f1, in0=pv[:, :, 0 : D // 2], in1=pv[:, :, D // 2 : D],
                    op=mybir.AluOpType.add,
                )
                nc.vector.tensor_tensor(
                    out=f2, in0=f1[:, :, 0 : D // 4], in1=f1[:, :, D // 4 : D // 2],
                    op=mybir.AluOpType.add,
                )
                score_t = score_pool.tile([128, un], F32)
                nc.vector.tensor_reduce(
                    out=score_t[:],
                    in_=f2,
                    axis=mybir.AxisListType.X,
                    op=mybir.AluOpType.add,
                )

                # score_rep[gc, n, d] = score[gc, n]  (ACT, bf16 out)
                srep_t = srep_pool.tile([128, FD], BF16)
                nc.scalar.copy(
                    out=srep_t[:].rearrange(
                        "p (gc n d) -> p gc n d", gc=GC, n=NN
                    ),
                    in_=score_t[:]
                    .rearrange("p (gc n) -> p gc n", gc=GC)
                    .unsqueeze(3)
                    .to_broadcast([128, GC, NN, D]),
                )

                # ent cast fp32 -> bf16 on ACT (enables DVE 2x wmul)
                entb_t = entb_pool.tile([128, FD], BF16)
                nc.scalar.copy(out=entb_t[:], in_=ent_t[:])

                # w[n, gc, d] = score_rep * ent, bf16 2x, in two n-halves so
                # the matmuls overlap the second half
                w_t = w_pool.tile([128, FD], BF16)
                psum_t = psum_pool.tile([128, GC * D], F32)
                out_t = out_pool.tile([128, GC * D], F32)
                wv = w_t[:].rearrange("p (n gc d) -> p n gc d", n=NN, gc=GC)
                ev = entb_t[:].rearrange("p (gc n d) -> p gc n d", gc=GC, n=NN)
                sv = srep_t[:].rearrange("p (gc n d) -> p gc n d", gc=GC, n=NN)
                half = NN // 2
                for h in range(2):
                    nlo, nhi = h * half, (h + 1) * half
                    nc.vector.tensor_tensor(
                        out=wv[:, nlo:nhi].rearrange("p n gc d -> p gc n d"),
                        in0=ev[:, :, nlo:nhi],
                        in1=sv[:, :, nlo:nhi],
                        op=mybir.AluOpType.mult,
                    )
                    for c in range(nlo, nhi):
                        nc.tensor.matmul(
                            out=psum_t[:],
                            lhsT=ident[:],
                            rhs=w_t[:, c * GC * D : (c + 1) * GC * D],
                            start=(c == 0),
                            stop=(c == NN - 1),
                        )

                nc.scalar.copy(out=out_t[:], in_=psum_t[:])
                nc.scalar.dma_start(out=out_v[pb, gt], in_=out_t[:])

    nc.compile()
    return nc


_NC_CACHE: dict = {}


def _get_nc(b_local: int = B_LOCAL):
    if b_local not in _NC_CACHE:
        _NC_CACHE[b_local] = _build_nc(b_local)
    return _NC_CACHE[b_local]


def run_sharded(drug, rel, ent, trace: bool = False):
    """Shard batch dim across the 8 cores, run, gather. Returns
    (full output [B, G, D], BassKernelResults)."""
    drug = np.ascontiguousarray(np.asarray(drug, dtype=np.float32))
    rel = np.ascontiguousarray(np.asarray(rel, dtype=np.float32))
    ent = np.ascontiguousarray(np.asarray(ent, dtype=np.float32))
    b = drug.shape[0]
    nb = b // N_CORES
    assert nb * N_CORES == b
    nc = _get_nc(nb)
    in_maps = [
        {
            "drug": np.ascontiguousarray(drug[i * nb : (i + 1) * nb]),
            "rel": np.ascontiguousarray(rel[i * nb : (i + 1) * nb]),
            "ent": np.ascontiguousarray(ent[i * nb : (i + 1) * nb]),
        }
        for i in range(N_CORES)
    ]
    last_exc = None
    for attempt in range(3):
        try:
            res = run_bass_kernel_spmd(nc, in_maps, list(range(N_CORES)), trace=trace)
            break
        except Exception as exc:  # transient device-unrecoverable states
            last_exc = exc
            import time

            time.sleep(10 * (attempt + 1))
    else:
        raise last_exc
    out = np.concatenate([res.results[i]["out"] for i in range(N_CORES)], axis=0)
    return out, res


def kernel(drug, rel, ent):
    out, _ = run_sharded(drug, rel, ent, trace=False)
    return out
